# revision 10
# baseline (speedup 1.0000x reference)
"""GarNet layer kernel for Trainium2 (8 NeuronCores, data-parallel over batch).

Math (per example b):
    w    = exp(-d_av^2)                      [V=128, S=16]
    hi   = w^T @ fi_v / V                    [S, N=64]
    out  = mean_V(w)[:, None] * hi           [S, N] -> flattened [S*N]

Implementation notes:
  - Batch B=4096 is sharded 512/core across 8 cores (pure data parallel).
  - Per example, one fp32 matmul: lhsT = w [V=128, S=16], rhs = fi
    augmented with a constant column of 1/V^2, so PSUM column N holds
    sum_V(w)/V^2 and the final output is just psum[:, :N] * psum[:, N]
    per partition (exactly the reference quantity).
  - Four examples share one PSUM bank at partition offsets {0,32,64,96}
    via tile_position col-tiling, so the epilogue runs on 128-partition
    tiles and the four matmuls overlap in distinct PE column groups.
"""

import numpy as np
from contextlib import ExitStack

import concourse.bass as bass
import concourse.tile as tile
from concourse import mybir
from concourse.bass_utils import run_bass_kernel_spmd

B, V, S, N = 4096, 128, 16, 64
NCORES = 8
BPC = B // NCORES            # examples per core
ONES_VAL = 1.0 / (V * V)     # exact power of two; folds /V^2 into the matmul


def split_multi_waits(nc):
    """The walrus build in this container rejects >1 embedded sem-wait per
    instruction ("Too many sync wait commands" in setupSyncWait). Hoist every
    multi-wait list onto single-wait EventSemaphore instructions immediately
    before the owner on the same engine — identical semantics, since engine
    streams are in order."""
    fn = nc.m.functions[0]
    for block in fn.blocks:
        insts = list(block.instructions)
        changed = False
        new = []
        for inst in insts:
            si = inst.sync_info
            waits = list(si.on_wait) if (si and si.on_wait) else []
            if len(waits) > 1:
                changed = True
                for w in waits:
                    ev = mybir.InstEventSemaphore(
                        name=nc.get_next_instruction_name(), ins=[], outs=[]
                    )
                    ev.engine = inst.engine
                    ev.sync_info = mybir.SyncInfo(on_wait=[w], on_update=[])
                    new.append(ev)
                ups = list(si.on_update) if si.on_update else []
                inst.sync_info = mybir.SyncInfo(on_wait=[], on_update=ups)
            new.append(inst)
        if changed:
            block.instructions = new


def build(bpc=BPC, e_chunk=32, name="garnet", split_waits=True):
    """Build the per-core Bass module for a shard of `bpc` examples.

    split_waits: apply the walrus multi-wait workaround (needed for HW
    compile; leave False for CoreSim, whose race detector doesn't know
    about post-hoc instructions).
    """
    assert bpc % e_chunk == 0 and e_chunk % 8 == 0
    nchunk = bpc // e_chunk
    G = e_chunk // 8   # psum groups (8 examples each) per chunk
    Q = e_chunk // 2   # w pairs per chunk

    nc = bass.Bass(name=name)
    fi = nc.dram_tensor("fi_v", (bpc, V, N), mybir.dt.float32, kind="ExternalInput")
    dav = nc.dram_tensor("d_av", (bpc, V, S), mybir.dt.float32, kind="ExternalInput")
    out = nc.dram_tensor("out", (bpc, S * N), mybir.dt.float32, kind="ExternalOutput")

    f32 = mybir.dt.float32
    with tile.TileContext(nc) as tc, ExitStack() as ctx:
        fipool = ctx.enter_context(tc.tile_pool(name="fipool", bufs=2))
        dpool = ctx.enter_context(tc.tile_pool(name="dpool", bufs=2))
        opool = ctx.enter_context(tc.tile_pool(name="opool", bufs=2))
        colpool = ctx.enter_context(tc.tile_pool(name="colpool", bufs=4))
        psum = ctx.enter_context(tc.tile_pool(name="psum", bufs=8, space="PSUM"))

        for c in range(nchunk):
            b0 = c * e_chunk
            # fi chunk -> [V, e, N+1]; col N = 1/V^2 for the wbar column
            fi_t = fipool.tile([128, e_chunk, N + 1], f32)
            nc.vector.memset(fi_t[:, :, N : N + 1], ONES_VAL)
            nc.sync.dma_start(
                out=fi_t[:, :, 0:N],
                in_=fi[b0 : b0 + e_chunk].rearrange("e v n -> v e n"),
            )
            # d chunk -> [V, pair, slot, S] with slot layout [w_2q, ZERO, w_2q+1];
            # then w = exp(-d^2) on the two w slots only (zeros stay zero).
            # Each matmul then takes a 32-wide lhsT: pair-even = (w_a, Z),
            # pair-odd = (Z, w_b). With PSUM accumulate (start only on the
            # bank's first matmul), the zero half writes/accumulates zeros, so
            # 8 examples pack one bank at rows 16*jj with no junk rows.
            d_t = dpool.tile([128, Q, 3, S], f32)
            nc.vector.memset(d_t[:, :, 1, :], 0.0)
            dsrc = dav[b0 : b0 + e_chunk].rearrange("(q t) v s -> t v q s", t=2)
            for t in range(2):
                nc.sync.dma_start(out=d_t[:, :, 2 * t, :], in_=dsrc[t])
                nc.vector.tensor_mul(
                    d_t[:, :, 2 * t, :], d_t[:, :, 2 * t, :], d_t[:, :, 2 * t, :]
                )
                nc.scalar.activation(
                    d_t[:, :, 2 * t, :],
                    d_t[:, :, 2 * t, :],
                    mybir.ActivationFunctionType.Exp,
                    scale=-1.0,
                )

            o_t = opool.tile([128, G, N], f32)
            for g in range(G):
                ps = psum.tile([128, N + 1], f32)
                for jj in range(8):
                    e = g * 8 + jj          # example within chunk
                    q, t = e // 2, e % 2    # pair index, parity
                    nc.tensor.matmul(
                        out=ps[32 * (jj // 2) : 32 * (jj // 2) + 32, :],
                        lhsT=d_t[:, q, t : t + 2, :],
                        rhs=fi_t[:, e, :],
                        start=(t == 0),
                        stop=(t == 1),
                        tile_position=(0, 32 * (jj // 2)),
                    )
                col = colpool.tile([128, 1], f32)
                nc.scalar.copy(col, ps[:, N : N + 1])
                nc.vector.tensor_scalar_mul(o_t[:, g, :], ps[:, 0:N], col)

            # partition p = 16*jj + s maps linearly to DRAM offset p*256B of
            # example b0+8g+jj -> one full-128-partition DMA per chunk.
            dst = out[b0 : b0 + e_chunk].rearrange(
                "(g jj) (s n) -> (jj s) g n", jj=8, s=S
            )
            nc.sync.dma_start(out=dst, in_=o_t)

    if split_waits:
        split_multi_waits(nc)
    return nc


_NC_CACHE = {}


def _get_nc():
    if "nc" not in _NC_CACHE:
        _NC_CACHE["nc"] = build()
    return _NC_CACHE["nc"]


def kernel(fi_v: np.ndarray, d_av: np.ndarray) -> np.ndarray:
    fi_v = np.ascontiguousarray(np.asarray(fi_v, dtype=np.float32))
    d_av = np.ascontiguousarray(np.asarray(d_av, dtype=np.float32))
    assert fi_v.shape == (B, V, N) and d_av.shape == (B, V, S)
    nc = _get_nc()
    in_maps = [
        {
            "fi_v": fi_v[c * BPC : (c + 1) * BPC],
            "d_av": d_av[c * BPC : (c + 1) * BPC],
        }
        for c in range(NCORES)
    ]
    res = run_bass_kernel_spmd(nc, in_maps, core_ids=list(range(NCORES)))
    return np.concatenate([res.results[c]["out"] for c in range(NCORES)], axis=0)
